# revision 6
# baseline (speedup 1.0000x reference)
"""Trainium2 Bass kernel for nn_ClassicalHybridClassifier.

Pipeline: conv1(5x5,s2) -> maxpool(2,s1) -> conv2(3x3,s2) -> maxpool(2,s1)
          -> fc1 [120,55815] -> fc2 -> fc3 -> qnn tanh stack -> RBF vs 8192
          train states -> [1,2] output.

Sharding: each of the 8 cores computes a horizontal band of the conv pipeline
(bands over the 61 pool2 output rows: 8,8,8,8,8,7,7,7) and the matching
contraction slice of fc1 (tensor-parallel over fc1's 55815 input dim, weights
restructured host-side to match the on-chip feature layout). One AllReduce of
the [10,120] fc1 partials; the tiny tail (fc2/fc3/qnn/RBF over all 8192 train
states) is replicated on every core.

The conv/fc1 pipeline runs in bf16 (fp32 PSUM accumulation): halves input DMA
bytes and doubles PE/DVE throughput. x is packed host-side with even/odd
column planes so all conv1 matmul operands are contiguous (dual-pump). Input
DMAs are chunked and issued in first-use order across several DGE queues so
conv1 starts as soon as its first image chunk lands. A dummy AllReduce issued
at kernel start warms the CC-core mesh setup path before the real one.
"""

import numpy as np
import ml_dtypes

import concourse.bass as bass
import concourse.mybir as mybir
import concourse.tile as tile
from concourse import bass_utils, bacc

F32 = mybir.dt.float32
BF16 = mybir.dt.bfloat16
AF = mybir.ActivationFunctionType
ALU = mybir.AluOpType
AX = mybir.AxisListType

N_CORES = 8
BANDS = [(0, 8), (8, 16), (16, 24), (24, 32), (32, 40), (40, 47), (47, 54), (54, 61)]

B = 10          # batch
XR = 43         # x rows per core (padded)
XC = 252        # x cols incl 1+1 zero pad (stored as even|odd planes of 126)
C1R = 20        # conv1 out rows per core (padded)
P1R = 19        # pool1 rows per core (padded)
C2R = 9         # conv2 out rows per core (padded)
NJ = 61         # pool2 / fc1 spatial columns
WCOL = 600      # w1 slab columns at the head of x2e/x3e

# conv1 N chunking over images (PSUM bank = 512 fp32)
C1_CHUNKS = [(0, 4), (4, 3), (7, 3)]     # (img0, nimg): 4*124=496, 3*124=372
C2_CHUNKS = [(0, 8), (8, 2)]             # 8*62=496, 2*62=124
P2_CHUNKS = [(0, 8), (8, 2)]             # over (img, 61): 488, 122
SH_CHUNKS = [(0, 4), (4, 4), (8, 2)]     # shift-mm chunks: even N (492, 492, 246)
WS_CHUNKS = [(0, 20), (20, 40), (40, 61)]


def _build_nc():
    nc = bacc.Bacc("TRN2", target_bir_lowering=False, debug=False,
                   num_devices=N_CORES)

    d = {}
    def din(name, shape, dt):
        d[name] = nc.dram_tensor(name, list(shape), dt, kind="ExternalInput").ap()

    din("x2", (87, WCOL + B * XC), BF16)   # w1a | c0+c1 rows + ones row (eo planes)
    din("x3", (43, WCOL + B * XC), BF16)   # w1b | c2 rows (eo planes)
    din("pack2", (120, 768), BF16)     # s1m | s2a | s2b | w2
    din("pack1", (128, 576), F32)      # small fc/tail tensors
    din("wslab", (120, NJ, 120), BF16)

    out_d = nc.dram_tensor("out", [1, 2], F32, kind="ExternalOutput").ap()
    warm_d = nc.dram_tensor("warm", [1, 4], F32, kind="ExternalOutput").ap()

    C0 = WCOL + 4 * XC      # chunk 0: w1 slab + imgs 0-3
    C1 = WCOL + 7 * XC      # chunk 1: imgs 4-6

    with tile.TileContext(nc) as tc:
        with (
            tc.tile_pool(name="sb", bufs=1) as sb,
            tc.tile_pool(name="dr", bufs=1, space="DRAM") as dr,
        ):
            # ---- warmup collective: exercise the CC mesh setup path early ----
            WARMUP_AR = False
            if WARMUP_AR:
                war_in = dr.tile([1, 4], F32)
                war_out = dr.tile([1, 4], F32, addr_space="Shared")
                wz = sb.tile([1, 4], F32)
                nc.vector.memset(wz[:], 0.0)
                nc.sync.dma_start(war_in[:], wz[:])
                nc.gpsimd.collective_compute(
                    "AllReduce", ALU.add,
                    replica_groups=[list(range(N_CORES))],
                    ins=[war_in.opt()], outs=[war_out.opt()])

            # ---- DMAs in, first-use order, spread across DGE queues ----
            x2e = sb.tile([87, WCOL + B * XC], BF16)
            x3e = sb.tile([43, WCOL + B * XC], BF16)
            pack2_t = sb.tile([120, 768], BF16)
            pack1_t = sb.tile([128, 576], F32)
            wslab_t = sb.tile([120, NJ, 120], BF16)
            nc.sync.dma_start(x2e[:, 0:C0], d["x2"][:, 0:C0])
            nc.sync.dma_start(x3e[:, 0:C0], d["x3"][:, 0:C0])
            nc.scalar.dma_start(x2e[:, C0:C1], d["x2"][:, C0:C1])
            nc.scalar.dma_start(x3e[:, C0:C1], d["x3"][:, C0:C1])
            nc.sync.dma_start(x2e[:, C1:], d["x2"][:, C1:])
            nc.sync.dma_start(x3e[:, C1:], d["x3"][:, C1:])
            nc.gpsimd.dma_start(pack2_t[:], d["pack2"][:])
            nc.scalar.dma_start(pack1_t[:], d["pack1"][:])
            for j0, j1 in WS_CHUNKS:
                nc.gpsimd.dma_start(wslab_t[:, j0:j1, :], d["wslab"][:, j0:j1, :])

            # even/odd plane views of x: [p, img, eo, 126]
            x_a = x2e[:, WCOL:].rearrange("p (i e c) -> p i e c", e=2, c=126)
            x_b = x3e[:, WCOL:].rearrange("p (i e c) -> p i e c", e=2, c=126)
            w1a_t = x2e[:, 0:WCOL].rearrange("p (k m) -> p k m", m=120)
            w1b_t = x3e[:, 0:WCOL].rearrange("p (k m) -> p k m", m=120)

            s1m_t = pack2_t[0:120, 0:114]
            s2a_t = pack2_t[0:120, 114:234]
            s2b_t = pack2_t[0:15, 234:354]
            w2f = pack2_t[0:115, 354:759]          # [115, 3*135] flat

            small = {
                "fc1b": pack1_t[0:120, 0:1],
                "w2fcT": pack1_t[0:120, 1:85],
                "fc2b": pack1_t[0:84, 85:86],
                "w3fcT": pack1_t[0:84, 86:87],
                "b3vec": pack1_t[0:B, 87:88],
                "wq1T": pack1_t[0:B, 88:108],
                "wq2T": pack1_t[0:20, 108:113],
                "idt10": pack1_t[0:B, 113:123],
                "kclsb": pack1_t[0:1, 123:125],
                "ts_r": pack1_t[:, 128:448].rearrange("p (a b) -> p a b", b=5),
                "kcls_r": pack1_t[:, 448:576].rearrange("p (a b) -> p a b", b=64),
            }

            # ---- PE warmup during input DMA ----
            with tc.tile_pool(name="ps_w", bufs=1, space="PSUM") as ps_w:
                wsc = sb.tile([128, 512], BF16)
                nc.vector.memset(wsc[:], 0.0)
                wps = ps_w.tile([128, 512], F32)
                for i in range(10):
                    nc.tensor.matmul(wps[:, 0:512], wsc[:, 0:128], wsc[:],
                                     start=(i == 0), stop=(i == 9))
                wout = sb.tile([1, 4], F32)
                nc.vector.tensor_copy(wout[:], wps[0:1, 0:4])
                nc.sync.dma_start(warm_d[:], wout[:])

            # ---- conv1 + pool1 ----
            Cs = sb.tile([120, B, 124], BF16)      # conv1 psum eviction
            Ch = sb.tile([120, B, 123], BF16)      # horizontal max
            V = sb.tile([128, B, 125], BF16)       # pool1 out, (py,ich) + ones@114

            with tc.tile_pool(name="ps_1", bufs=1, space="PSUM") as ps1:
                Cp = ps1.tile([120, 1536], F32)    # conv1 psum, 3 banks
                Sh = ps1.tile([114, 1536], F32)    # shifted Ch

                for ci, (i0, ni) in enumerate(C1_CHUNKS):
                    nmm = 10
                    k = 0
                    for kx in range(5):
                        eo, j0 = kx % 2, kx // 2
                        for grp in range(2):
                            xt, wt, kdim = ((x_a, w1a_t, 87) if grp == 0
                                            else (x_b, w1b_t, 43))
                            rhs = xt[0:kdim, i0:i0 + ni, eo, j0:j0 + 124]
                            nc.tensor.matmul(
                                Cp[:, ci * 512: ci * 512 + ni * 124],
                                wt[0:kdim, kx, :], rhs,
                                start=(k == 0), stop=(k == nmm - 1))
                            k += 1
                    cv = Cp[:, ci * 512: ci * 512 + ni * 124].rearrange(
                        "p (i x) -> p i x", x=124)
                    # evict on ACT, then horizontal pool max
                    nc.scalar.copy(Cs[:, i0:i0 + ni, :], cv)
                    nc.vector.tensor_max(Ch[:, i0:i0 + ni, :],
                                         Cs[:, i0:i0 + ni, 0:123],
                                         cv[:, :, 1:124])

                # vertical pool via partition-shift matmul; V = max(Ch,0,Sh)
                # (engine writes must start at partition 0/32/64/96, so the
                # ones row at p=114 is painted via a [96:128] memset first)
                nc.vector.memset(V[96:128, :, :], 1.0)
                nc.vector.memset(V[0:114, :, 0:1], 0.0)
                nc.vector.memset(V[0:114, :, 124:125], 0.0)
                for ci, (i0, ni) in enumerate(SH_CHUNKS):
                    nc.tensor.matmul(
                        Sh[:, ci * 512: ci * 512 + ni * 123],
                        s1m_t[:], Ch[:, i0:i0 + ni, :],
                        start=True, stop=True)
                    sv = Sh[:, ci * 512: ci * 512 + ni * 123].rearrange(
                        "p (i x) -> p i x", x=123)
                    nc.vector.scalar_tensor_tensor(
                        V[0:114, i0:i0 + ni, 1:124],
                        Ch[0:114, i0:i0 + ni, :], 0.0, sv[0:114, :, :],
                        op0=ALU.max, op1=ALU.max)

            # ---- conv2 + pool2 ----
            C2s_a = sb.tile([120, B, 62], BF16)
            C2s_b = sb.tile([15, B, 62], BF16)
            C2h_a = sb.tile([120, B, 61], BF16)
            C2h_b = sb.tile([15, B, 61], BF16)
            V2 = sb.tile([120, B, NJ], BF16)

            with tc.tile_pool(name="ps_2", bufs=1, space="PSUM") as ps2:
                C2a = ps2.tile([120, 1024], F32)
                C2b = ps2.tile([15, 1024], F32)
                Sh2 = ps2.tile([120, 1024], F32)

                for ci, (i0, ni) in enumerate(C2_CHUNKS):
                    for grp, (cp, m0, m1) in enumerate(
                            ((C2a, 0, 120), (C2b, 120, 135))):
                        for kxp in range(3):
                            rhs = V[0:115, i0:i0 + ni, kxp:kxp + 123:2]
                            nc.tensor.matmul(
                                cp[:, ci * 512: ci * 512 + ni * 62],
                                w2f[:, kxp * 135 + m0: kxp * 135 + m1], rhs,
                                start=(kxp == 0), stop=(kxp == 2))
                    for cp, cs, ch in ((C2a, C2s_a, C2h_a), (C2b, C2s_b, C2h_b)):
                        cv = cp[:, ci * 512: ci * 512 + ni * 62].rearrange(
                            "p (i x) -> p i x", x=62)
                        nc.scalar.copy(cs[:, i0:i0 + ni, :], cv)
                        # relu + horizontal pool (one PSUM operand)
                        nc.vector.scalar_tensor_tensor(
                            ch[:, i0:i0 + ni, :],
                            cs[:, i0:i0 + ni, 0:61], 0.0, cv[:, :, 1:62],
                            op0=ALU.max, op1=ALU.max)

                for ci, (i0, ni) in enumerate(P2_CHUNKS):
                    nc.tensor.matmul(
                        Sh2[:, ci * 512: ci * 512 + ni * 61],
                        s2a_t[:], C2h_a[:, i0:i0 + ni, :],
                        start=True, stop=False)
                    nc.tensor.matmul(
                        Sh2[:, ci * 512: ci * 512 + ni * 61],
                        s2b_t[:], C2h_b[:, i0:i0 + ni, :],
                        start=False, stop=True)
                    sv = Sh2[:, ci * 512: ci * 512 + ni * 61].rearrange(
                        "p (i x) -> p i x", x=61)
                    nc.vector.tensor_max(V2[:, i0:i0 + ni, :],
                                         C2h_a[:, i0:i0 + ni, :], sv)

            # ---- fc1 (bf16, tensor-parallel contraction) ----
            h = sb.tile([120, NJ, B], BF16)
            nc.vector.tensor_copy(h[:], V2[:].rearrange("p i j -> p j i"))

            fc1s = sb.tile([B, 120], F32)
            with tc.tile_pool(name="ps_3", bufs=1, space="PSUM") as ps3:
                fps = ps3.tile([B, 120], F32)
                for j in range(NJ):
                    nc.tensor.matmul(fps[:], h[:, j, :], wslab_t[:, j, :],
                                     start=(j == 0), stop=(j == NJ - 1))
                nc.vector.tensor_copy(fc1s[:], fps[:])

            # ---- AllReduce fc1 partials ----
            arin = dr.tile([B, 120], F32)
            arout = dr.tile([B, 120], F32, addr_space="Shared")
            nc.sync.dma_start(arin[:], fc1s[:])
            nc.gpsimd.collective_compute(
                "AllReduce", ALU.add,
                replica_groups=[list(range(N_CORES))],
                ins=[arin.opt()], outs=[arout.opt()])
            h1post = sb.tile([B, 120], F32)
            nc.sync.dma_start(h1post[:], arout[:])

            # ---- tail (replicated) ----
            h1 = sb.tile([120, B], F32)
            h2 = sb.tile([84, B], F32)
            h10 = sb.tile([B, 1], F32)
            s1 = sb.tile([20, 1], F32)
            fs_row = sb.tile([1, 5], F32)
            fsb = sb.tile([128, 5], F32)
            diff = sb.tile([128, 64, 5], F32)
            sq = sb.tile([128, 64, 5], F32)
            d2 = sb.tile([128, 64], F32)
            kxv = sb.tile([128, 64], F32)
            pr = sb.tile([128, 2, 64], F32)
            krw = sb.tile([128, 2], F32)
            ones_t = sb.tile([128, 1], F32)
            out_sb = sb.tile([1, 2], F32)
            nc.vector.memset(ones_t[:], 1.0)

            with tc.tile_pool(name="ps_4", bufs=1, space="PSUM") as ps4:
                tp = ps4.tile([120, B], F32)
                nc.tensor.transpose(tp[:], h1post[:], small["idt10"][:])
                nc.scalar.activation(h1[:], tp[:], AF.Relu,
                                     bias=small["fc1b"][:])

                p2 = ps4.tile([84, B], F32)
                nc.tensor.matmul(p2[:], small["w2fcT"][:], h1[:],
                                 start=True, stop=True)
                nc.scalar.activation(h2[:], p2[:], AF.Relu,
                                     bias=small["fc2b"][:])

                p3 = ps4.tile([B, 1], F32)
                nc.tensor.matmul(p3[:], h2[:], small["w3fcT"][:],
                                 start=True, stop=True)
                nc.scalar.activation(h10[:], p3[:], AF.Identity,
                                     bias=small["b3vec"][:])

                p4 = ps4.tile([20, 1], F32)
                nc.tensor.matmul(p4[:], small["wq1T"][:], h10[:],
                                 start=True, stop=True)
                nc.scalar.activation(s1[:], p4[:], AF.Tanh)

                p5 = ps4.tile([1, 5], F32)
                nc.tensor.matmul(p5[:], s1[:], small["wq2T"][:],
                                 start=True, stop=True)
                nc.scalar.activation(fs_row[:], p5[:], AF.Tanh)

                nc.gpsimd.partition_broadcast(fsb[:], fs_row[0:1, :])
                nc.vector.tensor_sub(
                    diff[:], small["ts_r"][:],
                    fsb[:].unsqueeze(1).broadcast_to([128, 64, 5]))
                nc.vector.tensor_mul(sq[:], diff[:], diff[:])
                nc.vector.reduce_sum(d2[:], sq[:], axis=AX.X)
                nc.scalar.activation(kxv[:], d2[:], AF.Exp, scale=-1.0)
                nc.vector.tensor_mul(
                    pr[:], small["kcls_r"][:],
                    kxv[:].unsqueeze(1).broadcast_to([128, 2, 64]))
                nc.vector.reduce_sum(krw[:], pr[:], axis=AX.X)

                p6 = ps4.tile([1, 2], F32)
                nc.tensor.matmul(p6[:], ones_t[:], krw[:],
                                 start=True, stop=True)
                nc.vector.tensor_add(out_sb[:], p6[:], small["kclsb"][:])

            nc.sync.dma_start(out_d[:], out_sb[:])

    nc.compile()
    return nc


def _prep_inputs(inputs):
    f32 = np.float32
    bf16 = ml_dtypes.bfloat16
    x = np.asarray(inputs["x"], f32)
    conv1_w = np.asarray(inputs["conv1_w"], f32)
    conv1_b = np.asarray(inputs["conv1_b"], f32)
    conv2_w = np.asarray(inputs["conv2_w"], f32)
    conv2_b = np.asarray(inputs["conv2_b"], f32)
    fc1_w = np.asarray(inputs["fc1_w"], f32)
    fc1_b = np.asarray(inputs["fc1_b"], f32)
    fc2_w = np.asarray(inputs["fc2_w"], f32)
    fc2_b = np.asarray(inputs["fc2_b"], f32)
    fc3_w = np.asarray(inputs["fc3_w"], f32)
    fc3_b = np.asarray(inputs["fc3_b"], f32)
    qnn_w1 = np.asarray(inputs["qnn_w1"], f32)
    qnn_w2 = np.asarray(inputs["qnn_w2"], f32)
    ts = np.asarray(inputs["train_states"], f32)
    kcls_w = np.asarray(inputs["kcls_w"], f32)
    kcls_b = np.asarray(inputs["kcls_b"], f32)

    pack1 = np.zeros((128, 576), f32)
    pack1[0:120, 0:1] = fc1_b.reshape(120, 1)
    pack1[0:120, 1:85] = fc2_w.T
    pack1[0:84, 85:86] = fc2_b.reshape(84, 1)
    pack1[0:84, 86:87] = fc3_w.T
    pack1[0:B, 87:88] = fc3_b[0]
    pack1[0:B, 88:108] = qnn_w1.T
    pack1[0:20, 108:113] = qnn_w2.T
    pack1[0:B, 113:123] = np.eye(B, dtype=f32)
    pack1[0:1, 123:125] = kcls_b.reshape(1, 2)
    pack1[:, 128:448] = ts.reshape(128, 320)
    pack1[:, 448:576] = kcls_w.reshape(2, 128, 64).transpose(1, 0, 2).reshape(128, 128)
    shared = {"pack1": pack1}

    fc1_w4 = fc1_w.reshape(120, 15, 61, 61)

    in_maps = []
    for a, b in BANDS:
        nb = b - a
        Y0 = 2 * a - 1          # conv1 row of y_loc 0 (also pool1 row of py_loc 0)
        X0 = 4 * a - 3          # x row of r_loc 0

        # x slabs: x2 = [c0 rows | c1 rows | ones], x3 = [c2 rows];
        # columns stored as even|odd planes so conv1 taps are contiguous
        xs = np.zeros((3, XR, B, XC), f32)
        r_lo = max(0, X0)
        r_hi = min(250, X0 + XR)
        xs[:, r_lo - X0: r_hi - X0, :, 1:251] = (
            x[:, :, r_lo:r_hi, :].transpose(1, 2, 0, 3))
        # P[2j+e] -> (e, j) planes: [c, r, B, 2, 126] -> [c, r, B, 2*126]
        xeo = xs.reshape(3, XR, B, 126, 2).transpose(0, 1, 2, 4, 3)
        x2 = np.concatenate(
            [xeo[0], xeo[1], np.ones((1, B, 2, 126), f32)], axis=0)
        x3 = xeo[2]

        # conv1 banded weights: K=(c, r_loc)+bias, M=(y_loc, och), per kx
        w1 = np.zeros((3, 43, 5, 120), f32)     # [c, r_loc, kx, m=(y_loc,och)]
        for y_loc in range(C1R):
            y = Y0 + y_loc
            if not (0 <= y <= 123):
                continue
            for ky in range(5):
                r_loc = 2 * y_loc + ky
                if r_loc >= XR:
                    continue
                for c in range(3):
                    w1[c, r_loc, :, y_loc * 6: y_loc * 6 + 6] = \
                        conv1_w[:, c, ky, :].T
        w1a = np.zeros((87, 5, 120), f32)
        w1a[0:43] = w1[0]
        w1a[43:86] = w1[1]
        w1a[86, 0, :] = np.tile(conv1_b, C1R)   # bias row, kx=0 only
        w1b = np.ascontiguousarray(w1[2])

        # conv2 banded weights: K=(py_loc, ich)+bias@114, M=(i2_loc, och2)
        w2 = np.zeros((115, 3, 135), f32)
        for i2_loc in range(C2R):
            i2 = a + i2_loc
            if i2 > 61:
                continue
            for kyp in range(3):
                py_loc = 2 * i2_loc + kyp
                py = Y0 + py_loc
                if py_loc >= P1R or not (0 <= py <= 122):
                    continue
                for ich in range(6):
                    q = py_loc * 6 + ich
                    m0 = i2_loc * 15
                    w2[q, :, m0:m0 + 15] = conv2_w[:, ich, kyp, :].T
        w2[114, 0, :] = np.tile(conv2_b, 9)     # bias row, kxp=0 only

        # partition-shift matrices
        s1m = np.zeros((120, 114), f32)
        for m in range(114):
            s1m[m + 6, m] = 1.0
        s2a = np.zeros((120, 120), f32)
        s2b = np.zeros((15, 120), f32)
        for m in range(105):
            s2a[m + 15, m] = 1.0
        for m in range(105, 120):
            s2b[m - 105, m] = 1.0

        # fc1 weight slab: [p=(i2_loc,och2), j, och1]
        wsl = np.zeros((8, 15, NJ, 120), f32)
        nrow = min(nb, 8)
        wsl[0:nrow] = fc1_w4[:, :, a:a + nrow, :].transpose(2, 1, 3, 0)
        wslab = wsl.reshape(120, NJ, 120).astype(bf16)

        pack2 = np.zeros((120, 768), f32)
        pack2[0:120, 0:114] = s1m
        pack2[0:120, 114:234] = s2a
        pack2[0:15, 234:354] = s2b
        pack2[0:115, 354:759] = w2.reshape(115, 405)

        x2e = np.concatenate([w1a.reshape(87, WCOL),
                              x2.reshape(87, B * XC)], axis=1).astype(bf16)
        x3e = np.concatenate([w1b.reshape(43, WCOL),
                              x3.reshape(43, B * XC)], axis=1).astype(bf16)
        m = dict(shared)
        m.update({"x2": np.ascontiguousarray(x2e),
                  "x3": np.ascontiguousarray(x3e),
                  "pack2": pack2.astype(bf16),
                  "wslab": np.ascontiguousarray(wslab)})
        in_maps.append(m)
    return in_maps


_NC_CACHE = None


def kernel(**inputs) -> np.ndarray:
    global _NC_CACHE
    if _NC_CACHE is None:
        _NC_CACHE = _build_nc()
    nc = _NC_CACHE
    in_maps = _prep_inputs(inputs)
    res = bass_utils.run_bass_kernel_spmd(
        nc, in_maps, core_ids=list(range(N_CORES)))
    return res.results[0]["out"]


# revision 8
# speedup vs baseline: 1.0995x; 1.0995x over previous
"""Trainium2 Bass kernel for nn_ClassicalHybridClassifier.

Pipeline: conv1(5x5,s2) -> maxpool(2,s1) -> conv2(3x3,s2) -> maxpool(2,s1)
          -> fc1 [120,55815] -> fc2 -> fc3 -> qnn tanh stack -> RBF vs 8192
          train states -> [1,2] output.

Sharding: each of the 8 cores computes a horizontal band of the conv pipeline
(bands over the 61 pool2 output rows: 8,8,8,8,8,7,7,7) and the matching
contraction slice of fc1 (tensor-parallel over fc1's 55815 input dim, weights
restructured host-side to match the on-chip feature layout). One AllReduce of
the [10,120] fc1 partials; the tiny tail (fc2/fc3/qnn/RBF over all 8192 train
states) is replicated on every core.

The conv/fc1 pipeline runs in bf16 (fp32 PSUM accumulation): halves input DMA
bytes and doubles PE/DVE throughput. x is packed host-side with even/odd
column planes so all conv1 matmul operands are contiguous (dual-pump). Input
DMAs are chunked and issued in first-use order across several DGE queues so
conv1 starts as soon as its first image chunk lands. A dummy AllReduce issued
at kernel start warms the CC-core mesh setup path before the real one.
"""

import numpy as np
import ml_dtypes

import concourse.bass as bass
import concourse.mybir as mybir
import concourse.tile as tile
from concourse import bass_utils, bacc

F32 = mybir.dt.float32
BF16 = mybir.dt.bfloat16
AF = mybir.ActivationFunctionType
ALU = mybir.AluOpType
AX = mybir.AxisListType

N_CORES = 8
BANDS = [(0, 8), (8, 16), (16, 24), (24, 32), (32, 40), (40, 47), (47, 54), (54, 61)]

B = 10          # batch
XR = 43         # x rows per core (padded)
XC = 252        # x cols incl 1+1 zero pad (stored as even|odd planes of 126)
C1R = 20        # conv1 out rows per core (padded)
P1R = 19        # pool1 rows per core (padded)
C2R = 9         # conv2 out rows per core (padded)
NJ = 61         # pool2 / fc1 spatial columns
WCOL = 600      # w1 slab columns at the head of x2e/x3e

# conv1 N chunking over images (PSUM bank = 512 fp32)
C1_CHUNKS = [(0, 4), (4, 3), (7, 3)]     # (img0, nimg): 4*124=496, 3*124=372
C2_CHUNKS = [(0, 8), (8, 2)]             # 8*62=496, 2*62=124
P2_CHUNKS = [(0, 8), (8, 2)]             # over (img, 61): 488, 122
SH_CHUNKS = [(0, 4), (4, 4), (8, 2)]     # shift-mm chunks: even N (492, 492, 246)
WS_CHUNKS = [(0, 20), (20, 40), (40, 61)]


def _build_nc():
    nc = bacc.Bacc("TRN2", target_bir_lowering=False, debug=False,
                   num_devices=N_CORES)

    d = {}
    def din(name, shape, dt):
        d[name] = nc.dram_tensor(name, list(shape), dt, kind="ExternalInput").ap()

    din("x2", (87, WCOL + B * XC), BF16)   # w1a | c0+c1 rows + ones row (eo planes)
    din("x3", (43, WCOL + B * XC), BF16)   # w1b | c2 rows (eo planes)
    din("pack2", (120, 768), BF16)     # s1m | s2a | s2b | w2
    din("pack1", (128, 576), F32)      # small fc/tail tensors
    din("wslab", (120, NJ, 120), BF16)

    out_d = nc.dram_tensor("out", [1, 2], F32, kind="ExternalOutput").ap()
    warm_d = nc.dram_tensor("warm", [1, 4], F32, kind="ExternalOutput").ap()

    C0 = WCOL + 4 * XC      # chunk 0: w1 slab + imgs 0-3
    C1 = WCOL + 7 * XC      # chunk 1: imgs 4-6

    with tile.TileContext(nc) as tc:
        with (
            tc.tile_pool(name="sb", bufs=1) as sb,
            tc.tile_pool(name="dr", bufs=1, space="DRAM") as dr,
        ):
            # ---- warmup collective: exercise the CC mesh setup path early ----
            WARMUP_AR = True
            if WARMUP_AR:
                war_in = dr.tile([1, 4], F32)
                war_out = dr.tile([1, 4], F32, addr_space="Shared")
                wz = sb.tile([1, 4], F32)
                nc.vector.memset(wz[:], 0.0)
                nc.sync.dma_start(war_in[:], wz[:])
                nc.gpsimd.collective_compute(
                    "AllReduce", ALU.add,
                    replica_groups=[list(range(N_CORES))],
                    ins=[war_in.opt()], outs=[war_out.opt()])

            # ---- DMAs in, first-use order, spread across DGE queues ----
            x2e = sb.tile([87, WCOL + B * XC], BF16)
            x3e = sb.tile([43, WCOL + B * XC], BF16)
            pack2_t = sb.tile([120, 768], BF16)
            pack1_t = sb.tile([128, 576], F32)
            wslab_t = sb.tile([120, NJ, 120], BF16)
            # SWDGE (gpsimd) spreads each transfer's descriptors across ~13
            # HW queues; HWDGE pins a transfer to one queue. Everything big
            # goes on gpsimd in first-use order.
            nc.gpsimd.dma_start(x2e[:, 0:C0], d["x2"][:, 0:C0])
            nc.gpsimd.dma_start(x3e[:, 0:C0], d["x3"][:, 0:C0])
            nc.gpsimd.dma_start(x2e[:, C0:], d["x2"][:, C0:])
            nc.gpsimd.dma_start(x3e[:, C0:], d["x3"][:, C0:])
            nc.gpsimd.dma_start(pack2_t[:], d["pack2"][:])
            nc.scalar.dma_start(pack1_t[:], d["pack1"][:])
            nc.gpsimd.dma_start(wslab_t[:], d["wslab"][:])

            # even/odd plane views of x: [p, img, eo, 126]
            x_a = x2e[:, WCOL:].rearrange("p (i e c) -> p i e c", e=2, c=126)
            x_b = x3e[:, WCOL:].rearrange("p (i e c) -> p i e c", e=2, c=126)
            w1a_t = x2e[:, 0:WCOL].rearrange("p (k m) -> p k m", m=120)
            w1b_t = x3e[:, 0:WCOL].rearrange("p (k m) -> p k m", m=120)

            s1m_t = pack2_t[0:120, 0:114]
            s2a_t = pack2_t[0:120, 114:234]
            s2b_t = pack2_t[0:15, 234:354]
            w2f = pack2_t[0:115, 354:759]          # [115, 3*135] flat

            small = {
                "fc1b": pack1_t[0:120, 0:1],
                "w2fcT": pack1_t[0:120, 1:85],
                "fc2b": pack1_t[0:84, 85:86],
                "w3fcT": pack1_t[0:84, 86:87],
                "b3vec": pack1_t[0:B, 87:88],
                "wq1T": pack1_t[0:B, 88:108],
                "wq2T": pack1_t[0:20, 108:113],
                "idt10": pack1_t[0:B, 113:123],
                "kclsb": pack1_t[0:1, 123:125],
                "ts_r": pack1_t[:, 128:448].rearrange("p (a b) -> p a b", b=5),
                "kcls_r": pack1_t[:, 448:576].rearrange("p (a b) -> p a b", b=64),
            }

            # ---- PE warmup during input DMA ----
            with tc.tile_pool(name="ps_w", bufs=1, space="PSUM") as ps_w:
                wsc = sb.tile([128, 512], BF16)
                nc.vector.memset(wsc[:], 0.0)
                wps = ps_w.tile([128, 512], F32)
                for i in range(10):
                    nc.tensor.matmul(wps[:, 0:512], wsc[:, 0:128], wsc[:],
                                     start=(i == 0), stop=(i == 9))
                wout = sb.tile([1, 4], F32)
                nc.vector.tensor_copy(wout[:], wps[0:1, 0:4])
                nc.sync.dma_start(warm_d[:], wout[:])

            # ---- conv1 + pool1 ----
            Cs = sb.tile([120, B, 124], BF16)      # conv1 psum eviction
            Ch = sb.tile([120, B, 123], BF16)      # horizontal max
            V = sb.tile([128, B, 125], BF16)       # pool1 out, (py,ich) + ones@114

            with tc.tile_pool(name="ps_1", bufs=1, space="PSUM") as ps1:
                Cp = ps1.tile([120, 1536], F32)    # conv1 psum, 3 banks
                Sh = ps1.tile([114, 1536], F32)    # shifted Ch

                for ci, (i0, ni) in enumerate(C1_CHUNKS):
                    nmm = 10
                    k = 0
                    for kx in range(5):
                        eo, j0 = kx % 2, kx // 2
                        for grp in range(2):
                            xt, wt, kdim = ((x_a, w1a_t, 87) if grp == 0
                                            else (x_b, w1b_t, 43))
                            rhs = xt[0:kdim, i0:i0 + ni, eo, j0:j0 + 124]
                            nc.tensor.matmul(
                                Cp[:, ci * 512: ci * 512 + ni * 124],
                                wt[0:kdim, kx, :], rhs,
                                start=(k == 0), stop=(k == nmm - 1))
                            k += 1
                    cv = Cp[:, ci * 512: ci * 512 + ni * 124].rearrange(
                        "p (i x) -> p i x", x=124)
                    # evict on ACT, then horizontal pool max
                    nc.scalar.copy(Cs[:, i0:i0 + ni, :], cv)
                    nc.vector.tensor_max(Ch[:, i0:i0 + ni, :],
                                         Cs[:, i0:i0 + ni, 0:123],
                                         cv[:, :, 1:124])

                # vertical pool via partition-shift matmul; V = max(Ch,0,Sh)
                # (engine writes must start at partition 0/32/64/96, so the
                # ones row at p=114 is painted via a [96:128] memset first)
                nc.vector.memset(V[96:128, :, :], 1.0)
                nc.vector.memset(V[0:114, :, 0:1], 0.0)
                nc.vector.memset(V[0:114, :, 124:125], 0.0)
                for ci, (i0, ni) in enumerate(SH_CHUNKS):
                    nc.tensor.matmul(
                        Sh[:, ci * 512: ci * 512 + ni * 123],
                        s1m_t[:], Ch[:, i0:i0 + ni, :],
                        start=True, stop=True)
                    sv = Sh[:, ci * 512: ci * 512 + ni * 123].rearrange(
                        "p (i x) -> p i x", x=123)
                    nc.vector.scalar_tensor_tensor(
                        V[0:114, i0:i0 + ni, 1:124],
                        Ch[0:114, i0:i0 + ni, :], 0.0, sv[0:114, :, :],
                        op0=ALU.max, op1=ALU.max)

            # ---- conv2 + pool2 ----
            C2s_a = sb.tile([120, B, 62], BF16)
            C2s_b = sb.tile([15, B, 62], BF16)
            C2h_a = sb.tile([120, B, 61], BF16)
            C2h_b = sb.tile([15, B, 61], BF16)
            V2 = sb.tile([120, B, NJ], BF16)

            with tc.tile_pool(name="ps_2", bufs=1, space="PSUM") as ps2:
                C2a = ps2.tile([120, 1024], F32)
                C2b = ps2.tile([15, 1024], F32)
                Sh2 = ps2.tile([120, 1024], F32)

                for ci, (i0, ni) in enumerate(C2_CHUNKS):
                    for grp, (cp, m0, m1) in enumerate(
                            ((C2a, 0, 120), (C2b, 120, 135))):
                        for kxp in range(3):
                            rhs = V[0:115, i0:i0 + ni, kxp:kxp + 123:2]
                            nc.tensor.matmul(
                                cp[:, ci * 512: ci * 512 + ni * 62],
                                w2f[:, kxp * 135 + m0: kxp * 135 + m1], rhs,
                                start=(kxp == 0), stop=(kxp == 2))
                    for cp, cs, ch in ((C2a, C2s_a, C2h_a), (C2b, C2s_b, C2h_b)):
                        cv = cp[:, ci * 512: ci * 512 + ni * 62].rearrange(
                            "p (i x) -> p i x", x=62)
                        nc.scalar.copy(cs[:, i0:i0 + ni, :], cv)
                        # relu + horizontal pool (one PSUM operand)
                        nc.vector.scalar_tensor_tensor(
                            ch[:, i0:i0 + ni, :],
                            cs[:, i0:i0 + ni, 0:61], 0.0, cv[:, :, 1:62],
                            op0=ALU.max, op1=ALU.max)

                for ci, (i0, ni) in enumerate(P2_CHUNKS):
                    nc.tensor.matmul(
                        Sh2[:, ci * 512: ci * 512 + ni * 61],
                        s2a_t[:], C2h_a[:, i0:i0 + ni, :],
                        start=True, stop=False)
                    nc.tensor.matmul(
                        Sh2[:, ci * 512: ci * 512 + ni * 61],
                        s2b_t[:], C2h_b[:, i0:i0 + ni, :],
                        start=False, stop=True)
                    sv = Sh2[:, ci * 512: ci * 512 + ni * 61].rearrange(
                        "p (i x) -> p i x", x=61)
                    nc.vector.tensor_max(V2[:, i0:i0 + ni, :],
                                         C2h_a[:, i0:i0 + ni, :], sv)

            # ---- fc1 (bf16, tensor-parallel contraction) ----
            h = sb.tile([120, NJ, B], BF16)
            nc.vector.tensor_copy(h[:], V2[:].rearrange("p i j -> p j i"))

            fc1s = sb.tile([B, 120], F32)
            with tc.tile_pool(name="ps_3", bufs=1, space="PSUM") as ps3:
                fps = ps3.tile([B, 120], F32)
                for j in range(NJ):
                    nc.tensor.matmul(fps[:], h[:, j, :], wslab_t[:, j, :],
                                     start=(j == 0), stop=(j == NJ - 1))
                nc.vector.tensor_copy(fc1s[:], fps[:])

            # ---- AllReduce fc1 partials ----
            arin = dr.tile([B, 120], F32)
            arout = dr.tile([B, 120], F32, addr_space="Shared")
            nc.sync.dma_start(arin[:], fc1s[:])
            nc.gpsimd.collective_compute(
                "AllReduce", ALU.add,
                replica_groups=[list(range(N_CORES))],
                ins=[arin.opt()], outs=[arout.opt()])
            h1post = sb.tile([B, 120], F32)
            nc.sync.dma_start(h1post[:], arout[:])

            # ---- tail (replicated) ----
            h1 = sb.tile([120, B], F32)
            h2 = sb.tile([84, B], F32)
            h10 = sb.tile([B, 1], F32)
            s1 = sb.tile([20, 1], F32)
            fs_row = sb.tile([1, 5], F32)
            fsb = sb.tile([128, 5], F32)
            diff = sb.tile([128, 64, 5], F32)
            sq = sb.tile([128, 64, 5], F32)
            d2 = sb.tile([128, 64], F32)
            kxv = sb.tile([128, 64], F32)
            pr = sb.tile([128, 2, 64], F32)
            krw = sb.tile([128, 2], F32)
            ones_t = sb.tile([128, 1], F32)
            out_sb = sb.tile([1, 2], F32)
            nc.vector.memset(ones_t[:], 1.0)

            with tc.tile_pool(name="ps_4", bufs=1, space="PSUM") as ps4:
                tp = ps4.tile([120, B], F32)
                nc.tensor.transpose(tp[:], h1post[:], small["idt10"][:])
                nc.scalar.activation(h1[:], tp[:], AF.Relu,
                                     bias=small["fc1b"][:])

                p2 = ps4.tile([84, B], F32)
                nc.tensor.matmul(p2[:], small["w2fcT"][:], h1[:],
                                 start=True, stop=True)
                nc.scalar.activation(h2[:], p2[:], AF.Relu,
                                     bias=small["fc2b"][:])

                p3 = ps4.tile([B, 1], F32)
                nc.tensor.matmul(p3[:], h2[:], small["w3fcT"][:],
                                 start=True, stop=True)
                nc.scalar.activation(h10[:], p3[:], AF.Identity,
                                     bias=small["b3vec"][:])

                p4 = ps4.tile([20, 1], F32)
                nc.tensor.matmul(p4[:], small["wq1T"][:], h10[:],
                                 start=True, stop=True)
                nc.scalar.activation(s1[:], p4[:], AF.Tanh)

                p5 = ps4.tile([1, 5], F32)
                nc.tensor.matmul(p5[:], s1[:], small["wq2T"][:],
                                 start=True, stop=True)
                nc.scalar.activation(fs_row[:], p5[:], AF.Tanh)

                nc.gpsimd.partition_broadcast(fsb[:], fs_row[0:1, :])
                nc.vector.tensor_sub(
                    diff[:], small["ts_r"][:],
                    fsb[:].unsqueeze(1).broadcast_to([128, 64, 5]))
                nc.vector.tensor_mul(sq[:], diff[:], diff[:])
                nc.vector.reduce_sum(d2[:], sq[:], axis=AX.X)
                nc.scalar.activation(kxv[:], d2[:], AF.Exp, scale=-1.0)
                nc.vector.tensor_mul(
                    pr[:], small["kcls_r"][:],
                    kxv[:].unsqueeze(1).broadcast_to([128, 2, 64]))
                nc.vector.reduce_sum(krw[:], pr[:], axis=AX.X)

                p6 = ps4.tile([1, 2], F32)
                nc.tensor.matmul(p6[:], ones_t[:], krw[:],
                                 start=True, stop=True)
                nc.vector.tensor_add(out_sb[:], p6[:], small["kclsb"][:])

            nc.sync.dma_start(out_d[:], out_sb[:])

    nc.compile()
    return nc


def _prep_inputs(inputs):
    f32 = np.float32
    bf16 = ml_dtypes.bfloat16
    x = np.asarray(inputs["x"], f32)
    conv1_w = np.asarray(inputs["conv1_w"], f32)
    conv1_b = np.asarray(inputs["conv1_b"], f32)
    conv2_w = np.asarray(inputs["conv2_w"], f32)
    conv2_b = np.asarray(inputs["conv2_b"], f32)
    fc1_w = np.asarray(inputs["fc1_w"], f32)
    fc1_b = np.asarray(inputs["fc1_b"], f32)
    fc2_w = np.asarray(inputs["fc2_w"], f32)
    fc2_b = np.asarray(inputs["fc2_b"], f32)
    fc3_w = np.asarray(inputs["fc3_w"], f32)
    fc3_b = np.asarray(inputs["fc3_b"], f32)
    qnn_w1 = np.asarray(inputs["qnn_w1"], f32)
    qnn_w2 = np.asarray(inputs["qnn_w2"], f32)
    ts = np.asarray(inputs["train_states"], f32)
    kcls_w = np.asarray(inputs["kcls_w"], f32)
    kcls_b = np.asarray(inputs["kcls_b"], f32)

    pack1 = np.zeros((128, 576), f32)
    pack1[0:120, 0:1] = fc1_b.reshape(120, 1)
    pack1[0:120, 1:85] = fc2_w.T
    pack1[0:84, 85:86] = fc2_b.reshape(84, 1)
    pack1[0:84, 86:87] = fc3_w.T
    pack1[0:B, 87:88] = fc3_b[0]
    pack1[0:B, 88:108] = qnn_w1.T
    pack1[0:20, 108:113] = qnn_w2.T
    pack1[0:B, 113:123] = np.eye(B, dtype=f32)
    pack1[0:1, 123:125] = kcls_b.reshape(1, 2)
    pack1[:, 128:448] = ts.reshape(128, 320)
    pack1[:, 448:576] = kcls_w.reshape(2, 128, 64).transpose(1, 0, 2).reshape(128, 128)
    shared = {"pack1": pack1}

    fc1_w4 = fc1_w.reshape(120, 15, 61, 61)

    in_maps = []
    for a, b in BANDS:
        nb = b - a
        Y0 = 2 * a - 1          # conv1 row of y_loc 0 (also pool1 row of py_loc 0)
        X0 = 4 * a - 3          # x row of r_loc 0

        # x slabs: x2 = [c0 rows | c1 rows | ones], x3 = [c2 rows];
        # columns stored as even|odd planes so conv1 taps are contiguous
        xs = np.zeros((3, XR, B, XC), f32)
        r_lo = max(0, X0)
        r_hi = min(250, X0 + XR)
        xs[:, r_lo - X0: r_hi - X0, :, 1:251] = (
            x[:, :, r_lo:r_hi, :].transpose(1, 2, 0, 3))
        # P[2j+e] -> (e, j) planes: [c, r, B, 2, 126] -> [c, r, B, 2*126]
        xeo = xs.reshape(3, XR, B, 126, 2).transpose(0, 1, 2, 4, 3)
        x2 = np.concatenate(
            [xeo[0], xeo[1], np.ones((1, B, 2, 126), f32)], axis=0)
        x3 = xeo[2]

        # conv1 banded weights: K=(c, r_loc)+bias, M=(y_loc, och), per kx
        w1 = np.zeros((3, 43, 5, 120), f32)     # [c, r_loc, kx, m=(y_loc,och)]
        for y_loc in range(C1R):
            y = Y0 + y_loc
            if not (0 <= y <= 123):
                continue
            for ky in range(5):
                r_loc = 2 * y_loc + ky
                if r_loc >= XR:
                    continue
                for c in range(3):
                    w1[c, r_loc, :, y_loc * 6: y_loc * 6 + 6] = \
                        conv1_w[:, c, ky, :].T
        w1a = np.zeros((87, 5, 120), f32)
        w1a[0:43] = w1[0]
        w1a[43:86] = w1[1]
        w1a[86, 0, :] = np.tile(conv1_b, C1R)   # bias row, kx=0 only
        w1b = np.ascontiguousarray(w1[2])

        # conv2 banded weights: K=(py_loc, ich)+bias@114, M=(i2_loc, och2)
        w2 = np.zeros((115, 3, 135), f32)
        for i2_loc in range(C2R):
            i2 = a + i2_loc
            if i2 > 61:
                continue
            for kyp in range(3):
                py_loc = 2 * i2_loc + kyp
                py = Y0 + py_loc
                if py_loc >= P1R or not (0 <= py <= 122):
                    continue
                for ich in range(6):
                    q = py_loc * 6 + ich
                    m0 = i2_loc * 15
                    w2[q, :, m0:m0 + 15] = conv2_w[:, ich, kyp, :].T
        w2[114, 0, :] = np.tile(conv2_b, 9)     # bias row, kxp=0 only

        # partition-shift matrices
        s1m = np.zeros((120, 114), f32)
        for m in range(114):
            s1m[m + 6, m] = 1.0
        s2a = np.zeros((120, 120), f32)
        s2b = np.zeros((15, 120), f32)
        for m in range(105):
            s2a[m + 15, m] = 1.0
        for m in range(105, 120):
            s2b[m - 105, m] = 1.0

        # fc1 weight slab: [p=(i2_loc,och2), j, och1]
        wsl = np.zeros((8, 15, NJ, 120), f32)
        nrow = min(nb, 8)
        wsl[0:nrow] = fc1_w4[:, :, a:a + nrow, :].transpose(2, 1, 3, 0)
        wslab = wsl.reshape(120, NJ, 120).astype(bf16)

        pack2 = np.zeros((120, 768), f32)
        pack2[0:120, 0:114] = s1m
        pack2[0:120, 114:234] = s2a
        pack2[0:15, 234:354] = s2b
        pack2[0:115, 354:759] = w2.reshape(115, 405)

        x2e = np.concatenate([w1a.reshape(87, WCOL),
                              x2.reshape(87, B * XC)], axis=1).astype(bf16)
        x3e = np.concatenate([w1b.reshape(43, WCOL),
                              x3.reshape(43, B * XC)], axis=1).astype(bf16)
        m = dict(shared)
        m.update({"x2": np.ascontiguousarray(x2e),
                  "x3": np.ascontiguousarray(x3e),
                  "pack2": pack2.astype(bf16),
                  "wslab": np.ascontiguousarray(wslab)})
        in_maps.append(m)
    return in_maps


_NC_CACHE = None


def kernel(**inputs) -> np.ndarray:
    global _NC_CACHE
    if _NC_CACHE is None:
        _NC_CACHE = _build_nc()
    nc = _NC_CACHE
    in_maps = _prep_inputs(inputs)
    res = bass_utils.run_bass_kernel_spmd(
        nc, in_maps, core_ids=list(range(N_CORES)))
    return res.results[0]["out"]


# revision 14
# speedup vs baseline: 1.1970x; 1.0887x over previous
"""Trainium2 Bass kernel for nn_ClassicalHybridClassifier.

Pipeline: conv1(5x5,s2) -> maxpool(2,s1) -> conv2(3x3,s2) -> maxpool(2,s1)
          -> fc1 [120,55815] -> fc2 -> fc3 -> qnn tanh stack -> RBF vs 8192
          train states -> [1,2] output.

Sharding: each of the 8 cores computes a horizontal band of the conv pipeline
(bands over the 61 pool2 output rows: 8,8,8,8,8,7,7,7) and the matching
contraction slice of fc1 (tensor-parallel over fc1's 55815 input dim, weights
restructured host-side to match the on-chip feature layout). One AllReduce of
the [10,120] fc1 partials; the tiny tail (fc2/fc3/qnn/RBF over all 8192 train
states) is replicated on every core.

The conv/fc1 pipeline runs in bf16 (fp32 PSUM accumulation): halves input DMA
bytes and doubles PE/DVE throughput. x is packed host-side with even/odd
column planes so all conv1 matmul operands are contiguous (dual-pump). Input
DMAs are chunked and issued in first-use order across several DGE queues so
conv1 starts as soon as its first image chunk lands. A dummy AllReduce issued
at kernel start warms the CC-core mesh setup path before the real one.
"""

import numpy as np
import ml_dtypes

import concourse.bass as bass
import concourse.mybir as mybir
import concourse.tile as tile
from concourse import bass_utils, bacc

F32 = mybir.dt.float32
BF16 = mybir.dt.bfloat16
AF = mybir.ActivationFunctionType
ALU = mybir.AluOpType
AX = mybir.AxisListType

N_CORES = 8
BANDS = [(0, 8), (8, 16), (16, 24), (24, 32), (32, 40), (40, 47), (47, 54), (54, 61)]

B = 10          # batch
XR = 43         # x rows per core (padded)
XC = 252        # x cols incl 1+1 zero pad (stored as even|odd planes of 126)
C1R = 20        # conv1 out rows per core (padded)
P1R = 19        # pool1 rows per core (padded)
C2R = 9         # conv2 out rows per core (padded)
NJ = 61         # pool2 / fc1 spatial columns
WCOL = 600      # w1 slab columns at the head of x2e/x3e

# conv1 N chunking over images (PSUM bank = 512 fp32)
C1_CHUNKS = [(0, 2), (2, 4), (6, 4)]     # (img0, nimg): 4*124=496, 3*124=372
C2_CHUNKS = [(0, 8), (8, 2)]             # 8*62=496, 2*62=124
P2_CHUNKS = [(0, 8), (8, 2)]             # over (img, 61): 488, 122
SH_CHUNKS = [(0, 4), (4, 4), (8, 2)]     # shift-mm chunks: even N (492, 492, 246)
WS_CHUNKS = [(0, 20), (20, 40), (40, 61)]


def _build_nc():
    nc = bacc.Bacc("TRN2", target_bir_lowering=False, debug=False,
                   num_devices=N_CORES)

    d = {}
    def din(name, shape, dt):
        d[name] = nc.dram_tensor(name, list(shape), dt, kind="ExternalInput").ap()

    din("x2", (87, WCOL + B * XC), BF16)   # w1a | c0+c1 rows + ones row (eo planes)
    din("x3", (43, WCOL + B * XC), BF16)   # w1b | c2 rows (eo planes)
    din("pack2", (120, 768), BF16)     # s1m | s2a | s2b | w2
    din("pack1", (128, 576), F32)      # small fc/tail tensors
    din("wslab", (120, NJ, 120), BF16)

    out_d = nc.dram_tensor("out", [1, 2], F32, kind="ExternalOutput").ap()
    warm_d = nc.dram_tensor("warm", [1, 4], F32, kind="ExternalOutput").ap()

    C0 = WCOL + 4 * XC      # chunk 0: w1 slab + imgs 0-3
    C1 = WCOL + 7 * XC      # chunk 1: imgs 4-6

    with tile.TileContext(nc) as tc:
        with (
            tc.tile_pool(name="sb", bufs=1) as sb,
            tc.tile_pool(name="dr", bufs=1, space="DRAM") as dr,
        ):
            # ---- warmup collective: exercise the CC mesh setup path early ----
            WARMUP_AR = False
            if WARMUP_AR:
                war_in = dr.tile([1, 4], F32)
                war_out = dr.tile([1, 4], F32, addr_space="Shared")
                wz = sb.tile([1, 4], F32)
                nc.vector.memset(wz[:], 0.0)
                nc.sync.dma_start(war_in[:], wz[:])
                nc.gpsimd.collective_compute(
                    "AllReduce", ALU.add,
                    replica_groups=[list(range(N_CORES))],
                    ins=[war_in.opt()], outs=[war_out.opt()])

            # ---- DMAs in, first-use order, spread across DGE queues ----
            x2e = sb.tile([87, WCOL + B * XC], BF16)
            x3e = sb.tile([43, WCOL + B * XC], BF16)
            pack2_t = sb.tile([120, 768], BF16)
            pack1_t = sb.tile([128, 576], F32)
            wslab_t = sb.tile([120, NJ, 120], BF16)
            # SWDGE (gpsimd) spreads each transfer's descriptors across ~13
            # HW queues; HWDGE pins a transfer to one queue. Everything big
            # goes on gpsimd in first-use order.
            nc.gpsimd.dma_start(x2e[:, 0:C0], d["x2"][:, 0:C0])
            nc.gpsimd.dma_start(x3e[:, 0:C0], d["x3"][:, 0:C0])
            nc.gpsimd.dma_start(x2e[:, C0:], d["x2"][:, C0:])
            nc.gpsimd.dma_start(x3e[:, C0:], d["x3"][:, C0:])
            nc.gpsimd.dma_start(pack2_t[:], d["pack2"][:])
            nc.scalar.dma_start(pack1_t[:], d["pack1"][:])
            nc.gpsimd.dma_start(wslab_t[:], d["wslab"][:])

            # even/odd plane views of x: [p, img, eo, 126]
            x_a = x2e[:, WCOL:].rearrange("p (i e c) -> p i e c", e=2, c=126)
            x_b = x3e[:, WCOL:].rearrange("p (i e c) -> p i e c", e=2, c=126)
            w1a_t = x2e[:, 0:WCOL].rearrange("p (k m) -> p k m", m=120)
            w1b_t = x3e[:, 0:WCOL].rearrange("p (k m) -> p k m", m=120)

            s1m_t = pack2_t[0:120, 0:114]
            s2a_t = pack2_t[0:120, 114:234]
            s2b_t = pack2_t[0:15, 234:354]
            w2f = pack2_t[0:115, 354:759]          # [115, 3*135] flat

            small = {
                "fc1b": pack1_t[0:120, 0:1],
                "w2fcT": pack1_t[0:120, 1:85],
                "fc2b": pack1_t[0:84, 85:86],
                "w3fcT": pack1_t[0:84, 86:87],
                "b3vec": pack1_t[0:B, 87:88],
                "wq1T": pack1_t[0:B, 88:108],
                "wq2T": pack1_t[0:20, 108:113],
                "idt10": pack1_t[0:B, 113:123],
                "kclsb": pack1_t[0:1, 123:125],
                "ts_r": pack1_t[:, 128:448].rearrange("p (a b) -> p a b", b=5),
                "kcls_r": pack1_t[:, 448:576].rearrange("p (a b) -> p a b", b=64),
            }

            # ---- PE warmup during input DMA ----
            with tc.tile_pool(name="ps_w", bufs=1, space="PSUM") as ps_w:
                wsc = sb.tile([128, 512], BF16)
                nc.vector.memset(wsc[:], 0.0)
                wps = ps_w.tile([128, 512], F32)
                for i in range(10):
                    nc.tensor.matmul(wps[:, 0:512], wsc[:, 0:128], wsc[:],
                                     start=(i == 0), stop=(i == 9))
                wout = sb.tile([1, 4], F32)
                nc.vector.tensor_copy(wout[:], wps[0:1, 0:4])
                nc.sync.dma_start(warm_d[:], wout[:])

            # ---- conv1 + pool1 ----
            Cs = sb.tile([120, B, 124], BF16)      # conv1 psum eviction
            Ch = sb.tile([120, B, 123], BF16)      # horizontal max
            V = sb.tile([128, B, 125], BF16)       # pool1 out, (py,ich) + ones@114

            with tc.tile_pool(name="ps_1", bufs=1, space="PSUM") as ps1:
                Cp = ps1.tile([120, 1536], F32)    # conv1 psum, 3 banks
                Sh = ps1.tile([114, 1536], F32)    # shifted Ch

                for ci, (i0, ni) in enumerate(C1_CHUNKS):
                    nmm = 10
                    k = 0
                    for grp in range(2):
                        for kx in range(5):
                            eo, j0 = kx % 2, kx // 2
                            xt, wt, kdim = ((x_a, w1a_t, 87) if grp == 0
                                            else (x_b, w1b_t, 43))
                            rhs = xt[0:kdim, i0:i0 + ni, eo, j0:j0 + 124]
                            nc.tensor.matmul(
                                Cp[:, ci * 512: ci * 512 + ni * 124],
                                wt[0:kdim, kx, :], rhs,
                                start=(k == 0), stop=(k == nmm - 1))
                            k += 1
                    cv = Cp[:, ci * 512: ci * 512 + ni * 124].rearrange(
                        "p (i x) -> p i x", x=124)
                    # evict on ACT, then horizontal pool max
                    nc.scalar.copy(Cs[:, i0:i0 + ni, :], cv)
                    nc.vector.tensor_max(Ch[:, i0:i0 + ni, :],
                                         Cs[:, i0:i0 + ni, 0:123],
                                         cv[:, :, 1:124])

                # vertical pool via partition-shift matmul; V = max(Ch,0,Sh)
                # (engine writes must start at partition 0/32/64/96, so the
                # ones row at p=114 is painted via a [96:128] memset first)
                nc.vector.memset(V[96:128, :, :], 1.0)
                nc.vector.memset(V[0:114, :, 0:1], 0.0)
                nc.vector.memset(V[0:114, :, 124:125], 0.0)
                for ci, (i0, ni) in enumerate(SH_CHUNKS):
                    nc.tensor.matmul(
                        Sh[:, ci * 512: ci * 512 + ni * 123],
                        s1m_t[:], Ch[:, i0:i0 + ni, :],
                        start=True, stop=True)
                    sv = Sh[:, ci * 512: ci * 512 + ni * 123].rearrange(
                        "p (i x) -> p i x", x=123)
                    nc.vector.scalar_tensor_tensor(
                        V[0:114, i0:i0 + ni, 1:124],
                        Ch[0:114, i0:i0 + ni, :], 0.0, sv[0:114, :, :],
                        op0=ALU.max, op1=ALU.max)

            # ---- conv2 + pool2 ----
            C2s_a = sb.tile([120, B, 62], BF16)
            C2s_b = sb.tile([15, B, 62], BF16)
            C2h_a = sb.tile([120, B, 61], BF16)
            C2h_b = sb.tile([15, B, 61], BF16)
            V2 = sb.tile([120, B, NJ], BF16)

            with tc.tile_pool(name="ps_2", bufs=1, space="PSUM") as ps2:
                C2a = ps2.tile([120, 1024], F32)
                C2b = ps2.tile([15, 1024], F32)
                Sh2 = ps2.tile([120, 1024], F32)

                for ci, (i0, ni) in enumerate(C2_CHUNKS):
                    for grp, (cp, m0, m1) in enumerate(
                            ((C2a, 0, 120), (C2b, 120, 135))):
                        for kxp in range(3):
                            rhs = V[0:115, i0:i0 + ni, kxp:kxp + 123:2]
                            nc.tensor.matmul(
                                cp[:, ci * 512: ci * 512 + ni * 62],
                                w2f[:, kxp * 135 + m0: kxp * 135 + m1], rhs,
                                start=(kxp == 0), stop=(kxp == 2))
                    for cp, cs, ch in ((C2a, C2s_a, C2h_a), (C2b, C2s_b, C2h_b)):
                        cv = cp[:, ci * 512: ci * 512 + ni * 62].rearrange(
                            "p (i x) -> p i x", x=62)
                        nc.scalar.copy(cs[:, i0:i0 + ni, :], cv)
                        # relu + horizontal pool (one PSUM operand)
                        nc.vector.scalar_tensor_tensor(
                            ch[:, i0:i0 + ni, :],
                            cs[:, i0:i0 + ni, 0:61], 0.0, cv[:, :, 1:62],
                            op0=ALU.max, op1=ALU.max)

                # pool2 chunked over j (not images) so fc1 can start on the
                # first j-range while the second is still pooling
                h = sb.tile([120, NJ, B], BF16)
                for ci, (j0, j1) in enumerate(((0, 31), (31, 61))):
                    nj = j1 - j0
                    nc.tensor.matmul(
                        Sh2[:, ci * 512: ci * 512 + B * nj],
                        s2a_t[:], C2h_a[:, :, j0:j1],
                        start=True, stop=False)
                    nc.tensor.matmul(
                        Sh2[:, ci * 512: ci * 512 + B * nj],
                        s2b_t[:], C2h_b[:, :, j0:j1],
                        start=False, stop=True)
                    sv = Sh2[:, ci * 512: ci * 512 + B * nj].rearrange(
                        "p (i x) -> p i x", x=nj)
                    nc.vector.tensor_max(V2[:, :, j0:j1],
                                         C2h_a[:, :, j0:j1], sv)
                    nc.vector.tensor_copy(
                        h[:, j0:j1, :],
                        V2[:, :, j0:j1].rearrange("p i j -> p j i"))

            # ---- fc1 (bf16, tensor-parallel contraction) ----
            fc1s = sb.tile([B, 120], F32)
            with tc.tile_pool(name="ps_3", bufs=1, space="PSUM") as ps3:
                fps = ps3.tile([B, 120], F32)
                for j in range(NJ):
                    nc.tensor.matmul(fps[:], h[:, j, :], wslab_t[:, j, :],
                                     start=(j == 0), stop=(j == NJ - 1))
                nc.vector.tensor_copy(fc1s[:], fps[:])

            # ---- AllReduce fc1 partials ----
            arin = dr.tile([B, 120], F32)
            arout = dr.tile([B, 120], F32, addr_space="Shared")
            nc.sync.dma_start(arin[:], fc1s[:])
            nc.gpsimd.collective_compute(
                "AllReduce", ALU.add,
                replica_groups=[list(range(N_CORES))],
                ins=[arin.opt()], outs=[arout.opt()])
            h1post = sb.tile([B, 120], F32)
            nc.sync.dma_start(h1post[:], arout[:])

            # ---- tail (replicated) ----
            h1 = sb.tile([120, B], F32)
            h2 = sb.tile([84, B], F32)
            h10 = sb.tile([B, 1], F32)
            s1 = sb.tile([20, 1], F32)
            fs_row = sb.tile([1, 5], F32)
            fsb = sb.tile([128, 5], F32)
            prod = sb.tile([128, 64, 5], F32)
            sqf = sb.tile([1, 5], F32)
            s2f = sb.tile([1, 1], F32)
            ef = sb.tile([1, 1], F32)
            d2 = sb.tile([128, 64], F32)
            kxv = sb.tile([128, 64], F32)
            pr = sb.tile([128, 2, 64], F32)
            krw = sb.tile([128, 2], F32)
            ones_t = sb.tile([128, 1], F32)
            out_sb = sb.tile([1, 2], F32)
            nc.vector.memset(ones_t[:], 1.0)

            with tc.tile_pool(name="ps_4", bufs=1, space="PSUM") as ps4:
                tp = ps4.tile([120, B], F32)
                nc.tensor.transpose(tp[:], h1post[:], small["idt10"][:])
                nc.scalar.activation(h1[:], tp[:], AF.Relu,
                                     bias=small["fc1b"][:])

                p2 = ps4.tile([84, B], F32)
                nc.tensor.matmul(p2[:], small["w2fcT"][:], h1[:],
                                 start=True, stop=True)
                nc.scalar.activation(h2[:], p2[:], AF.Relu,
                                     bias=small["fc2b"][:])

                p3 = ps4.tile([B, 1], F32)
                nc.tensor.matmul(p3[:], h2[:], small["w3fcT"][:],
                                 start=True, stop=True)
                nc.scalar.activation(h10[:], p3[:], AF.Identity,
                                     bias=small["b3vec"][:])

                p4 = ps4.tile([20, 1], F32)
                nc.tensor.matmul(p4[:], small["wq1T"][:], h10[:],
                                 start=True, stop=True)
                nc.scalar.activation(s1[:], p4[:], AF.Tanh)

                p5 = ps4.tile([1, 5], F32)
                nc.tensor.matmul(p5[:], s1[:], small["wq2T"][:],
                                 start=True, stop=True)
                nc.scalar.activation(fs_row[:], p5[:], AF.Tanh)

                # exp(-|fs - ts|^2) = exp(-|fs|^2) * exp(2 fs.ts) * exp(-|ts|^2);
                # the last factor is folded into kcls_r host-side.
                nc.gpsimd.partition_broadcast(fsb[:], fs_row[0:1, :])
                nc.vector.tensor_mul(sqf[:], fs_row[:], fs_row[:])
                nc.vector.reduce_sum(s2f[:], sqf[:], axis=AX.X)
                nc.scalar.activation(ef[:], s2f[:], AF.Exp, scale=-1.0)
                nc.vector.tensor_mul(
                    prod[:], small["ts_r"][:],
                    fsb[:].unsqueeze(1).broadcast_to([128, 64, 5]))
                nc.vector.reduce_sum(d2[:], prod[:], axis=AX.X)
                nc.scalar.activation(kxv[:], d2[:], AF.Exp, scale=2.0)
                nc.vector.tensor_mul(
                    pr[:], small["kcls_r"][:],
                    kxv[:].unsqueeze(1).broadcast_to([128, 2, 64]))
                nc.vector.reduce_sum(krw[:], pr[:], axis=AX.X)

                p6 = ps4.tile([1, 2], F32)
                nc.tensor.matmul(p6[:], ones_t[:], krw[:],
                                 start=True, stop=True)
                nc.vector.scalar_tensor_tensor(
                    out_sb[:], p6[:], ef[:], small["kclsb"][:],
                    op0=ALU.mult, op1=ALU.add)

            nc.sync.dma_start(out_d[:], out_sb[:])

    nc.compile()
    return nc


def _prep_inputs(inputs):
    f32 = np.float32
    bf16 = ml_dtypes.bfloat16
    x = np.asarray(inputs["x"], f32)
    conv1_w = np.asarray(inputs["conv1_w"], f32)
    conv1_b = np.asarray(inputs["conv1_b"], f32)
    conv2_w = np.asarray(inputs["conv2_w"], f32)
    conv2_b = np.asarray(inputs["conv2_b"], f32)
    fc1_w = np.asarray(inputs["fc1_w"], f32)
    fc1_b = np.asarray(inputs["fc1_b"], f32)
    fc2_w = np.asarray(inputs["fc2_w"], f32)
    fc2_b = np.asarray(inputs["fc2_b"], f32)
    fc3_w = np.asarray(inputs["fc3_w"], f32)
    fc3_b = np.asarray(inputs["fc3_b"], f32)
    qnn_w1 = np.asarray(inputs["qnn_w1"], f32)
    qnn_w2 = np.asarray(inputs["qnn_w2"], f32)
    ts = np.asarray(inputs["train_states"], f32)
    kcls_w = np.asarray(inputs["kcls_w"], f32)
    kcls_b = np.asarray(inputs["kcls_b"], f32)

    pack1 = np.zeros((128, 576), f32)
    pack1[0:120, 0:1] = fc1_b.reshape(120, 1)
    pack1[0:120, 1:85] = fc2_w.T
    pack1[0:84, 85:86] = fc2_b.reshape(84, 1)
    pack1[0:84, 86:87] = fc3_w.T
    pack1[0:B, 87:88] = fc3_b[0]
    pack1[0:B, 88:108] = qnn_w1.T
    pack1[0:20, 108:113] = qnn_w2.T
    pack1[0:B, 113:123] = np.eye(B, dtype=f32)
    pack1[0:1, 123:125] = kcls_b.reshape(1, 2)
    pack1[:, 128:448] = ts.reshape(128, 320)
    kclsq = kcls_w * np.exp(-np.sum(ts * ts, axis=1))[None, :]
    pack1[:, 448:576] = kclsq.reshape(2, 128, 64).transpose(1, 0, 2).reshape(128, 128)
    shared = {"pack1": pack1}

    fc1_w4 = fc1_w.reshape(120, 15, 61, 61)

    in_maps = []
    for a, b in BANDS:
        nb = b - a
        Y0 = 2 * a - 1          # conv1 row of y_loc 0 (also pool1 row of py_loc 0)
        X0 = 4 * a - 3          # x row of r_loc 0

        # x slabs: x2 = [c0 rows | c1 rows | ones], x3 = [c2 rows];
        # columns stored as even|odd planes so conv1 taps are contiguous
        xs = np.zeros((3, XR, B, XC), f32)
        r_lo = max(0, X0)
        r_hi = min(250, X0 + XR)
        xs[:, r_lo - X0: r_hi - X0, :, 1:251] = (
            x[:, :, r_lo:r_hi, :].transpose(1, 2, 0, 3))
        # P[2j+e] -> (e, j) planes: [c, r, B, 2, 126] -> [c, r, B, 2*126]
        xeo = xs.reshape(3, XR, B, 126, 2).transpose(0, 1, 2, 4, 3)
        x2 = np.concatenate(
            [xeo[0], xeo[1], np.ones((1, B, 2, 126), f32)], axis=0)
        x3 = xeo[2]

        # conv1 banded weights: K=(c, r_loc)+bias, M=(y_loc, och), per kx
        w1 = np.zeros((3, 43, 5, 120), f32)     # [c, r_loc, kx, m=(y_loc,och)]
        for y_loc in range(C1R):
            y = Y0 + y_loc
            if not (0 <= y <= 123):
                continue
            for ky in range(5):
                r_loc = 2 * y_loc + ky
                if r_loc >= XR:
                    continue
                for c in range(3):
                    w1[c, r_loc, :, y_loc * 6: y_loc * 6 + 6] = \
                        conv1_w[:, c, ky, :].T
        w1a = np.zeros((87, 5, 120), f32)
        w1a[0:43] = w1[0]
        w1a[43:86] = w1[1]
        w1a[86, 0, :] = np.tile(conv1_b, C1R)   # bias row, kx=0 only
        w1b = np.ascontiguousarray(w1[2])

        # conv2 banded weights: K=(py_loc, ich)+bias@114, M=(i2_loc, och2)
        w2 = np.zeros((115, 3, 135), f32)
        for i2_loc in range(C2R):
            i2 = a + i2_loc
            if i2 > 61:
                continue
            for kyp in range(3):
                py_loc = 2 * i2_loc + kyp
                py = Y0 + py_loc
                if py_loc >= P1R or not (0 <= py <= 122):
                    continue
                for ich in range(6):
                    q = py_loc * 6 + ich
                    m0 = i2_loc * 15
                    w2[q, :, m0:m0 + 15] = conv2_w[:, ich, kyp, :].T
        w2[114, 0, :] = np.tile(conv2_b, 9)     # bias row, kxp=0 only

        # partition-shift matrices
        s1m = np.zeros((120, 114), f32)
        for m in range(114):
            s1m[m + 6, m] = 1.0
        s2a = np.zeros((120, 120), f32)
        s2b = np.zeros((15, 120), f32)
        for m in range(105):
            s2a[m + 15, m] = 1.0
        for m in range(105, 120):
            s2b[m - 105, m] = 1.0

        # fc1 weight slab: [p=(i2_loc,och2), j, och1]
        wsl = np.zeros((8, 15, NJ, 120), f32)
        nrow = min(nb, 8)
        wsl[0:nrow] = fc1_w4[:, :, a:a + nrow, :].transpose(2, 1, 3, 0)
        wslab = wsl.reshape(120, NJ, 120).astype(bf16)

        pack2 = np.zeros((120, 768), f32)
        pack2[0:120, 0:114] = s1m
        pack2[0:120, 114:234] = s2a
        pack2[0:15, 234:354] = s2b
        pack2[0:115, 354:759] = w2.reshape(115, 405)

        x2e = np.concatenate([w1a.reshape(87, WCOL),
                              x2.reshape(87, B * XC)], axis=1).astype(bf16)
        x3e = np.concatenate([w1b.reshape(43, WCOL),
                              x3.reshape(43, B * XC)], axis=1).astype(bf16)
        m = dict(shared)
        m.update({"x2": np.ascontiguousarray(x2e),
                  "x3": np.ascontiguousarray(x3e),
                  "pack2": pack2.astype(bf16),
                  "wslab": np.ascontiguousarray(wslab)})
        in_maps.append(m)
    return in_maps


_NC_CACHE = None


def kernel(**inputs) -> np.ndarray:
    global _NC_CACHE
    if _NC_CACHE is None:
        _NC_CACHE = _build_nc()
    nc = _NC_CACHE
    in_maps = _prep_inputs(inputs)
    res = bass_utils.run_bass_kernel_spmd(
        nc, in_maps, core_ids=list(range(N_CORES)))
    return res.results[0]["out"]
